# revision 7
# baseline (speedup 1.0000x reference)
import sys
sys.path.insert(0, '/opt/trn_rl_repo')
import numpy as np

P = 128
NCORES = 8
F = 128
SLICE = 12544          # real rows per core (98 blocks)
NBLK = 98
SLICE_PAD = 12672      # + 128-row zero block
QBLK = [25, 25, 24, 24]            # dest blocks per quarter
QB0 = [0, 25, 50, 74, 98]          # block boundaries
QROWS = [3200, 3200, 3072, 3200]   # rows per core per quarter (q3 incl 128 zero)
QREAL = [3200, 3200, 3072, 3072]   # real rows per core per quarter
QROW0 = [0, 3200, 6400, 9472]      # start row of quarter within slice
TROWS = [8 * r for r in QROWS]     # rows per range table (<= 25600 < 32767)
TGATH = 4096                        # gather tile rows (multiple of 128)


def _bf16():
    import ml_dtypes
    return ml_dtypes.bfloat16


def _build_bass(K, Lg, NCH, choffG, blkoff, weights, biases, Wp, bp):
    """K: [NBLK][4] chunks per (block, range); Lg[4] stream rows per range;
    NCH total chunks; choffG[4] chunk-column offset per range; blkoff[g][b]
    chunk offset of block b within range-g stream."""
    from concourse import bass, bacc, mybir
    import concourse.tile as tile

    nc = bacc.Bacc(num_devices=NCORES, num_swdge_queues=4)
    bf = mybir.dt.bfloat16
    f32 = mybir.dt.float32

    x_in = nc.declare_dram_parameter("x_in", [SLICE_PAD, F], bf, isOutput=False)
    midx_in = [nc.declare_dram_parameter(f"midx{g}", [P, Lg[g] // 16], mybir.dt.int16, isOutput=False)
               for g in range(4)]
    mdlc_in = nc.declare_dram_parameter("mdlc", [P, NCH], f32, isOutput=False)
    mnrm_in = nc.declare_dram_parameter("mnrm", [P, NCH], f32, isOutput=False)
    y_out = nc.declare_dram_parameter("y_out", [P, 256], f32, isOutput=True)

    # internal DRAM: staging for AG inputs + gathered tables
    xq_in = [nc.dram_tensor(f"xq{q}", [QROWS[q], F], bf) for q in range(4)]
    h_q = [[nc.dram_tensor(f"hq{i}_{q}", [QROWS[q], F], bf) for q in range(4)]
           for i in range(3)]
    vq = [[nc.dram_tensor(f"vq{i}_{q}", [TROWS[q], F], bf, addr_space="Shared")
           for q in range(4)] for i in range(4)]

    iota_np = np.broadcast_to(np.arange(P, dtype=np.float32), (P, P)).astype(_bf16())
    iota_d = nc.inline_tensor(np.asarray(iota_np), name="iota_c")
    W_d = [nc.inline_tensor(np.ascontiguousarray(w.astype(_bf16())), name=f"W{i}")
           for i, w in enumerate(weights)]
    B_d = [nc.inline_tensor(np.broadcast_to(b.astype(np.float32), (P, F)).copy(), name=f"B{i}")
           for i, b in enumerate(biases)]
    Wp_d = [nc.inline_tensor(np.ascontiguousarray(Wp[i * F:(i + 1) * F, :].astype(_bf16())), name=f"Wp{i}")
            for i in range(3)]
    bpd = float(bp[0] - bp[1])
    bpd_d = nc.inline_tensor(np.full((P, 1), bpd, np.float32), name="bpd_c")

    AF = mybir.ActivationFunctionType
    ALU = mybir.AluOpType
    rg = [list(range(NCORES))]
    TCH = TGATH // 128          # chunks per full gather tile
    ntile = [(Lg[g] + TGATH - 1) // TGATH for g in range(4)]

    with tile.TileContext(nc) as tc:
        with (
            tc.tile_pool(name="const", bufs=1) as cpool,
            tc.tile_pool(name="msg", bufs=3) as mpool,
            tc.tile_pool(name="work", bufs=4) as pool,
            tc.tile_pool(name="epi", bufs=1) as epool,
            tc.tile_pool(name="psum", bufs=2, space="PSUM") as psum,
            tc.tile_pool(name="psum2", bufs=2, space="PSUM") as psum2,
        ):
            # ---- prologue ----
            # stage x quarters + allgather round-0 tables
            for q in range(4):
                nc.sync.dma_start(out=xq_in[q][:, :], in_=x_in[QROW0[q]:QROW0[q] + QROWS[q], :])
                nc.gpsimd.collective_compute(
                    "AllGather", ALU.bypass, replica_groups=rg,
                    ins=[xq_in[q][:].opt()], outs=[vq[0][q][:].opt()],
                )
            # zero rows of future h tables (q3 zero block)
            ztile = cpool.tile([P, F], bf)
            nc.vector.memset(ztile[:], 0.0)
            for i in range(3):
                nc.sync.dma_start(out=h_q[i][3][QREAL[3]:QROWS[3], :], in_=ztile[:])

            iota_t = cpool.tile([P, P], bf)
            nc.sync.dma_start(out=iota_t[:], in_=iota_d[:, :])
            bpd_t = cpool.tile([P, 1], f32)
            nc.sync.dma_start(out=bpd_t[:], in_=bpd_d[:, :])
            W_t, B_t, Wp_t = [], [], []
            for i in range(3):
                wt = cpool.tile([P, F], bf, tag=f"w{i}")
                nc.sync.dma_start(out=wt[:], in_=W_d[i][:, :])
                W_t.append(wt)
                bt = cpool.tile([P, F], f32, tag=f"b{i}")
                nc.sync.dma_start(out=bt[:], in_=B_d[i][:, :])
                B_t.append(bt)
                wpt = cpool.tile([P, 2], bf, tag=f"wp{i}")
                nc.sync.dma_start(out=wpt[:], in_=Wp_d[i][:, :])
                Wp_t.append(wpt)

            midx_t = []
            for g in range(4):
                mt = cpool.tile([P, Lg[g] // 16], mybir.dt.int16, tag=f"midx{g}")
                nc.sync.dma_start(out=mt[:], in_=midx_in[g][:, :])
                midx_t.append(mt)
            mdlc_t = cpool.tile([P, NCH], f32)
            nc.sync.dma_start(out=mdlc_t[:], in_=mdlc_in[:, :])
            mnrm_t = cpool.tile([P, NCH], f32)
            nc.sync.dma_start(out=mnrm_t[:], in_=mnrm_in[:, :])

            yA = cpool.tile([P, 256], f32)
            nc.vector.memset(yA[:], 0.0)
            hsl = cpool.tile([P, SLICE], bf)

            qn = [0]

            # ---- 4 propagation rounds ----
            for i in range(4):
                cur_tile = [[-1, None] for _ in range(4)]  # per range: tile idx, handle

                def get_msg(g, pos_chunks, i=i, cur_tile=cur_tile):
                    t_idx = (pos_chunks * 128) // TGATH
                    if cur_tile[g][0] != t_idx:
                        rows0 = t_idx * TGATH
                        n_i = min(TGATH, Lg[g] - rows0)
                        mt = mpool.tile([P, TGATH], bf, tag=f"msg{g}")
                        nc.gpsimd.dma_gather(
                            out_ap=mt[:, :n_i].rearrange("p (c e) -> p c e", e=128),
                            in_ap=vq[i][g][:, :],
                            idxs_ap=midx_t[g][:, rows0 // 16:(rows0 + n_i) // 16],
                            num_idxs=n_i,
                            num_idxs_reg=n_i,
                            elem_size=128,
                            single_packet=False,
                            queue_num=qn[0] % 4,
                        )
                        qn[0] += 1
                        cur_tile[g][0] = t_idx
                        cur_tile[g][1] = mt
                    within = pos_chunks - cur_tile[g][0] * TCH
                    return cur_tile[g][1][:, within * 128:(within + 1) * 128]

                for b in range(NBLK):
                    totb = sum(K[b][g] for g in range(4))
                    gt = psum.tile([P, P], f32, tag="gt", space="PSUM")
                    done = 0
                    for g in range(4):
                        for k in range(K[b][g]):
                            pos = blkoff[g][b] + k
                            msg = get_msg(g, pos)
                            col = choffG[g] + pos
                            S = pool.tile([P, P], bf, tag="S")
                            nc.vector.tensor_scalar(
                                out=S[:], in0=iota_t[:],
                                scalar1=mdlc_t[:, col:col + 1], op0=ALU.is_equal,
                                scalar2=mnrm_t[:, col:col + 1], op1=ALU.mult,
                            )
                            nc.tensor.matmul(out=gt[:], lhsT=msg, rhs=S[:],
                                             start=(done == 0), stop=(done == totb - 1))
                            done += 1
                    gts = pool.tile([P, P], bf, tag="gts")
                    nc.scalar.copy(out=gts[:], in_=gt[:])
                    if i < 3:
                        hp = psum2.tile([P, P], f32, tag="hp", space="PSUM")
                        nc.tensor.matmul(out=hp[:], lhsT=gts[:], rhs=W_t[i][:],
                                         start=True, stop=True)
                        hb = hsl[:, b * P:(b + 1) * P]
                        nc.vector.tensor_tensor(out=hb, in0=hp[:], in1=B_t[i][:], op=ALU.add)
                        nc.vector.tensor_scalar_max(out=hb, in0=hb, scalar1=0.0)
                    if i >= 1:
                        ypT = psum2.tile([P, 2], f32, tag="ypT", space="PSUM")
                        nc.tensor.matmul(out=ypT[:], lhsT=gts[:], rhs=Wp_t[i - 1][:],
                                         start=True, stop=True)
                        nc.vector.tensor_tensor(out=yA[:, b:b + 1], in0=yA[:, b:b + 1],
                                                in1=ypT[:, 0:1], op=ALU.add)
                        nc.vector.tensor_tensor(out=yA[:, 128 + b:129 + b], in0=yA[:, 128 + b:129 + b],
                                                in1=ypT[:, 1:2], op=ALU.add)
                    if i < 3:
                        for q in range(4):
                            if b == QB0[q + 1] - 1:
                                nb_q = QREAL[q] // 128
                                c0 = QB0[q] * P
                                nc.sync.dma_start(
                                    out=h_q[i][q][0:QREAL[q], :].rearrange("(b d) o -> d b o", d=P),
                                    in_=hsl[:, c0:c0 + nb_q * P].rearrange("d (b o) -> d b o", o=P))
                                nc.gpsimd.collective_compute(
                                    "AllGather", ALU.bypass, replica_groups=rg,
                                    ins=[h_q[i][q][:].opt()], outs=[vq[i + 1][q][:].opt()],
                                )

            # ---- epilogue: y = softmax over 2 logits = sigmoid(y0-y1+bp0-bp1) ----
            dif = epool.tile([P, NBLK], f32, tag="dif")
            nc.vector.tensor_tensor(out=dif[:], in0=yA[:, 0:NBLK], in1=yA[:, 128:128 + NBLK],
                                    op=ALU.subtract)
            sig = epool.tile([P, NBLK], f32, tag="sig")
            nc.scalar.activation(out=sig[:], in_=dif[:], func=AF.Sigmoid, bias=bpd_t[:])
            om = epool.tile([P, NBLK], f32, tag="om")
            nc.vector.tensor_scalar(out=om[:], in0=sig[:],
                                    scalar1=-1.0, op0=ALU.mult,
                                    scalar2=1.0, op1=ALU.add)
            nc.sync.dma_start(out=y_out[:, 0:NBLK], in_=sig[:])
            nc.sync.dma_start(out=y_out[:, 128:128 + NBLK], in_=om[:])

    nc.compile()
    return nc


def _prep(x, edge_index):
    """Host-side: gcn norm, per-core dest-block/src-range sorted + padded
    edge streams, int16 gather indices, chunk metadata."""
    bf16 = _bf16()
    N = x.shape[0]
    E = edge_index.shape[1]
    row = np.concatenate([edge_index[0].astype(np.int64), np.arange(N, dtype=np.int64)])
    col = np.concatenate([edge_index[1].astype(np.int64), np.arange(N, dtype=np.int64)])
    deg = np.bincount(col, minlength=N).astype(np.float32)
    dis = np.where(deg > 0, 1.0 / np.sqrt(deg), 0.0).astype(np.float32)
    nrm = dis[row] * dis[col]

    qi = np.repeat(np.arange(4), [25, 25, 24, 24])      # block -> quarter
    QR = np.array(QROWS); Q0 = np.array(QROW0)

    c_d = col // SLICE
    s_d = col % SLICE
    bl = s_d // P
    dlc = (s_d % P).astype(np.float32)
    c_s = row // SLICE
    s_s = row % SLICE
    g = qi[s_s // P]
    loc = (c_s * QR[g] + s_s - Q0[g]).astype(np.int16)

    key = (c_d * NBLK + bl) * 4 + g
    order = np.argsort(key, kind='stable')
    key_s = key[order]; loc_s = loc[order]; dlc_s = dlc[order]; nrm_s = nrm[order]

    ngrp = NCORES * NBLK * 4
    cnt = np.bincount(key_s, minlength=ngrp).reshape(NCORES, NBLK, 4)
    K = np.ceil(cnt.max(axis=0) / P).astype(np.int64)   # [NBLK, 4] unified
    # rank within each (c, bl, g) group
    gstart = np.zeros(ngrp + 1, np.int64)
    np.cumsum(cnt.reshape(-1), out=gstart[1:])
    rank = np.arange(key_s.shape[0], dtype=np.int64) - gstart[key_s]

    # chunk offset of block b within range-g stream (in chunks), and stream len
    blkoff = np.zeros((4, NBLK), np.int64)
    for gg in range(4):
        blkoff[gg, 1:] = np.cumsum(K[:-1, gg])
    Lg = [int(K[:, gg].sum()) * P for gg in range(4)]
    choffG = np.zeros(4, np.int64)
    choffG[1:] = np.cumsum([Lg[gg] // P for gg in range(3)])
    NCH = int(K.sum())

    # scatter each core's edges into padded per-range streams
    pos = blkoff[g[order], bl[order]] * P + rank        # position within (core, range) stream
    metas = []
    for c in range(NCORES):
        msk = (key_s // (NBLK * 4)) == c
        midx, mdlc, mnrm = [], [], []
        for gg in range(4):
            L = Lg[gg]
            ar = np.arange(L, dtype=np.int64)
            # pad slots: any in-bounds row works (S column is zero there);
            # spread over the first 128 rows of the table
            idxs = (ar % P).astype(np.int16)
            dl = np.zeros(L, np.float32)
            nr = np.zeros(L, np.float32)
            m2 = msk & (g[order] == gg)
            p2 = pos[m2]
            idxs[p2] = loc_s[m2]
            dl[p2] = dlc_s[m2]
            nr[p2] = nrm_s[m2]
            w = idxs.reshape(L // 16, 16).T.copy()       # [16, L/16]
            midx.append(np.tile(w, (8, 1)))              # [128, L/16]
            mdlc.append(dl.reshape(L // P, P).T)         # [128, L/P]
            mnrm.append(nr.reshape(L // P, P).T)
        mdlc = np.concatenate(mdlc, axis=1).astype(np.float32)
        mnrm = np.concatenate(mnrm, axis=1).astype(np.float32)
        metas.append((midx, np.ascontiguousarray(mdlc), np.ascontiguousarray(mnrm)))

    x_pad = np.zeros((NCORES, SLICE_PAD, F), np.float32)
    xr = x.astype(np.float32)
    for c in range(NCORES):
        lo = c * SLICE
        hi = min((c + 1) * SLICE, N)
        x_pad[c, :hi - lo] = xr[lo:hi]
    x_bf = x_pad.astype(bf16)

    return metas, x_bf, K, Lg, NCH, choffG, blkoff


LAST_RESULTS = None
LAST_NC = None
LAST_IN_MAPS = None


def kernel(x, edge_index, W0, b0, W1, b1, W2, b2, Wp, bp):
    global LAST_RESULTS, LAST_NC, LAST_IN_MAPS
    import os
    from concourse.bass_utils import run_bass_kernel_spmd

    x = np.asarray(x, dtype=np.float32)
    edge_index = np.asarray(edge_index)
    N = x.shape[0]

    metas, x_bf, K, Lg, NCH, choffG, blkoff = _prep(x, edge_index)

    nc = _build_bass(
        K, Lg, NCH, choffG, blkoff,
        [np.asarray(W0), np.asarray(W1), np.asarray(W2)],
        [np.asarray(b0), np.asarray(b1), np.asarray(b2)],
        np.asarray(Wp), np.asarray(bp),
    )

    in_maps = []
    for c in range(NCORES):
        midx, mdlc, mnrm = metas[c]
        m = {"x_in": x_bf[c], "mdlc": mdlc, "mnrm": mnrm}
        for g in range(4):
            m[f"midx{g}"] = np.ascontiguousarray(midx[g])
        in_maps.append(m)

    trace = bool(os.environ.get("KERNEL_TRACE"))
    res = run_bass_kernel_spmd(nc, in_maps, list(range(NCORES)), trace=trace)
    LAST_RESULTS = res
    LAST_NC = nc
    LAST_IN_MAPS = in_maps

    out = np.zeros((NCORES * SLICE, 2), np.float32)
    for c in range(NCORES):
        yo = res.results[c]["y_out"]              # [128, 256]
        y0 = yo[:, 0:NBLK].T.reshape(SLICE)       # [b,d] -> node b*128+d
        y1 = yo[:, 128:128 + NBLK].T.reshape(SLICE)
        out[c * SLICE:(c + 1) * SLICE, 0] = y0
        out[c * SLICE:(c + 1) * SLICE, 1] = y1
    return out[:N]
